# revision 2
# baseline (speedup 1.0000x reference)
"""GATv2 Bass kernel v3 for Trainium2, 8 NeuronCores.

Problem: B=2, N=512, FIN=128, H=4, D=64 GATv2 attention (dense graph).
Sharding: one (batch, head) pair per core (B*H = 8 = n_cores).

Math per (b, h):
  h[n, d]  = x[n, :] @ W_proj[h] (+ b_proj)
  zi = h @ W1^T, zj = h @ W2^T          (W1|W2 = W_cat_weight halves)
  score[i, j] = sum_e a_e * lrelu(zi[i,e] + zj[j,e] + bc[e])
  attn = softmax_j(score);  out[i, :] = attn[i, :] @ h

v3 design ("transposed scores, TensorE reduction"):
  Work in the transposed score layout scoresT[j, i]. The e-reduction is a
  TensorE matmul instead of DVE tensor_reduce:
    - contraction layout packs 32 j's x 4 e's: partition p = jl*4 + el.
      e-chunk c covers e = 4c..4c+3; j-group g covers {j : j % 16 == g}
      (strided groups keep the ybias DMA scatter contiguous), jl = j // 16.
    - ziT4[c][p, i] = ziT[4c + p%4, i] via a replication matmul (one-hot
      stationary REP_c), built once per chunk, reused by all 16 groups.
    - evac (c, g) applies the nonlinearity with the Y-add as the engine's
      per-partition bias/scalar operand, one [128, 512] op -> L fp16:
        ScalarE chunks: L = lrelu_0.2(ziT4 + Y)   (AF.Lrelu, alpha=0.2)
        DVE chunks:     L = relu(ziT4 + Y)        (tensor_scalar add+max)
      a*lrelu(u) = (1-slope)*a*relu(u) + slope*a*u: the relu chunks put
      0.8*a_e in the reduce stationary; the linear term's i-part is
      constant per row i (drops from softmax_j) and its j-part enters the
      Exp bias B_j.
    - reduce matmul (c, g): scoresT rows 32q..32q+32 of tile t (g = 4t+q)
      accumulate stat_c^T @ L over the 16 chunks.
  Softmax stays transposed: exp(scoresT + B_j) -> expT fp16 (no row-max;
  scores are bounded), zsum via ones-stationary matmul over j partitions,
  1/zsum folded into the output scale. Epilogue needs no transposes:
  out[i-block] = sum_t (expT[t] slice)^T @ h[t], h built row-interleaved to
  match j(t, p) = 16*jl + 4t + q.
"""

import numpy as np

import concourse.bacc as bacc
import concourse.mybir as mybir
import concourse.tile as tile
from concourse.bass_utils import run_bass_kernel_spmd

F32 = mybir.dt.float32
F16 = mybir.dt.float16
F16NP = np.float16

B, N, FIN, H, D = 2, 512, 128, 4, 64
E = D
NEG_SLOPE = 0.2

NC_CHUNK = 16     # e-chunks (4 e's each)
NG = 16           # j-groups (32 j's each, strided: j % 16 == g)
NB = N // 128     # 4 i-blocks / scoresT tiles

# chunk -> engine. 'S': ScalarE abs (lrelu = 0.6u + 0.4|u|); 'D': DVE relu
# (lrelu = 0.2u + 0.8relu(u)). Interleaved, DVE-heavy (DVE op is faster).
MODE = tuple('S' if c in (0, 3, 6, 9, 12) else 'D' for c in range(NC_CHUNK))

last_results = None
_cache = {}


def _build(use_bproj, use_bias_param):
    nc = bacc.Bacc("TRN2", target_bir_lowering=False, debug=False, num_devices=8)

    AF = mybir.ActivationFunctionType
    ALU = mybir.AluOpType

    x_d = nc.dram_tensor("x", [N, FIN], F32, kind="ExternalInput")
    m1t_d = nc.dram_tensor("m1t", [FIN, E], F32, kind="ExternalInput")
    m2t_d = nc.dram_tensor("m2t", [FIN, E], F32, kind="ExternalInput")
    zib_d = nc.dram_tensor("zib", [E, 1], F32, kind="ExternalInput")
    ybb_d = nc.dram_tensor("ybb", [E, 1], F32, kind="ExternalInput")
    amask_d = nc.dram_tensor("amask", [E, 1], F32, kind="ExternalInput")
    wp_d = nc.dram_tensor("wp", [FIN, D], F32, kind="ExternalInput")
    rep_d = nc.dram_tensor("rep", [E, NC_CHUNK * 128], F16, kind="ExternalInput")
    stat_d = nc.dram_tensor("stat", [128, NC_CHUNK * 32], F16, kind="ExternalInput")
    id128_d = nc.dram_tensor("id128", [128, 128], F32, kind="ExternalInput")
    ones_d = nc.dram_tensor("ones", [128, 1], F16, kind="ExternalInput")
    one11_d = nc.dram_tensor("one11", [1, 1], F32, kind="ExternalInput")
    if use_bproj:
        bpr_d = nc.dram_tensor("bprojrep", [128, D], F32, kind="ExternalInput")
    if use_bias_param:
        bprm_d = nc.dram_tensor("biasprm", [128, D], F32, kind="ExternalInput")
    out_d = nc.dram_tensor("out", [N, D], F32, kind="ExternalOutput")
    # ybias scatter scratch (dram roundtrip); host ignores these outputs
    scr1_d = nc.dram_tensor("scr1", [NC_CHUNK, 4, 32, NG], F32,
                            kind="ExternalOutput")
    scr2_d = nc.dram_tensor("scr2", [32, 4, NC_CHUNK, NG], F32,
                            kind="ExternalOutput")

    with tile.TileContext(nc) as tc:
        with tc.tile_pool(name="sb", bufs=1) as sb:
            xb = sb.tile([128, NB * 128], F32)
            xT = sb.tile([128, N], F32)
            m1t = sb.tile([FIN, E], F32)
            m2t = sb.tile([FIN, E], F32)
            zib = sb.tile([E, 1], F32)
            ybb = sb.tile([E, 1], F32)
            amask = sb.tile([E, 1], F32)
            wp = sb.tile([FIN, D], F32)
            rep = sb.tile([E, NC_CHUNK * 128], F16)
            stat = sb.tile([128, NC_CHUNK * 32], F16)
            id128 = sb.tile([128, 128], F32)
            ones = sb.tile([128, 1], F16)
            one11 = sb.tile([1, 1], F32)
            ziT = sb.tile([E, N], F16)
            yT = sb.tile([E, N], F32)
            ybias = sb.tile([128, NC_CHUNK * NG], F32)
            Bsb = sb.tile([128, NB], F32)
            h16 = [sb.tile([128, D], F16, tag=f"h{t}", name=f"h{t}")
                   for t in range(NB)]
            expT = [sb.tile([128, N], F16, tag=f"e{t}", name=f"e{t}")
                    for t in range(NB)]
            zsum_sb = sb.tile([1, N], F32)
            rz = sb.tile([128, NB], F32)
            yTsel = sb.tile([E, NB * 128], F32)
            xTsel = sb.tile([128, NB * 128], F32)
            if use_bproj:
                bpr = sb.tile([128, D], F32)
            if use_bias_param:
                bprm = sb.tile([128, D], F32)

            # ---------- input DMAs ----------
            for nb in range(NB):
                nc.sync.dma_start(xb[:, nb * 128:(nb + 1) * 128],
                                  x_d.ap()[nb * 128:(nb + 1) * 128, :])
            nc.sync.dma_start(m1t[:], m1t_d.ap())
            nc.sync.dma_start(m2t[:], m2t_d.ap())
            nc.sync.dma_start(zib[:], zib_d.ap())
            nc.sync.dma_start(ybb[:], ybb_d.ap())
            nc.sync.dma_start(amask[:], amask_d.ap())
            nc.sync.dma_start(wp[:], wp_d.ap())
            nc.sync.dma_start(rep[:], rep_d.ap())
            nc.sync.dma_start(stat[:], stat_d.ap())
            nc.sync.dma_start(id128[:], id128_d.ap())
            nc.sync.dma_start(ones[:], ones_d.ap())
            nc.sync.dma_start(one11[:], one11_d.ap())
            if use_bproj:
                nc.sync.dma_start(bpr[:], bpr_d.ap())
            if use_bias_param:
                nc.sync.dma_start(bprm[:], bprm_d.ap())

            # interleaved j view: j(t, p=32q+jl) = 16*jl + 4t + q
            def jsel(src, t):
                # src [P, 512] -> [P, 4, 32] view with free = (q, jl) so the
                # flat free index is p = 32q + jl <-> j = 16*jl + 4t + q
                v = src[:].rearrange("p (jl g) -> p g jl", g=NG)
                return v[:, 4 * t:4 * t + 4, :]

            # ---------- prep ----------
            with tc.tile_pool(name="pp", bufs=4, space="PSUM") as pp:
                # xT = x^T via PE transposes
                for nb in range(NB):
                    t = pp.tile([128, 512], F32, tag="t")
                    nc.tensor.transpose(t[:, 0:128],
                                        xb[:, nb * 128:(nb + 1) * 128], id128[:])
                    nc.scalar.copy(xT[:, nb * 128:(nb + 1) * 128], t[:, 0:128])
                # ziT (fp16) and yT (f32)
                t = pp.tile([128, 512], F32, tag="t")
                nc.tensor.matmul(t[0:E, :], m1t[:], xT[:], start=True, stop=True)
                nc.scalar.activation(ziT[:], t[0:E, :], AF.Identity,
                                     bias=zib[:, 0:1])
                t2 = pp.tile([128, 512], F32, tag="t")
                nc.tensor.matmul(t2[0:E, :], m2t[:], xT[:], start=True, stop=True)
                nc.scalar.activation(yT[:], t2[0:E, :], AF.Identity,
                                     bias=ybb[:, 0:1])
                # ybias scatter: yT -> scr1 -> (4 parallel permutes) scr2
                # -> ybias. dram isn't tile-tracked: chain with fan-out keys.
                hop1 = nc.sync.dma_start(
                    scr1_d.ap().rearrange("c el jl g -> (c el) (jl g)"), yT[:])
                for el in range(4):
                    tc.chain_iter_dep(f"yb_el{el}", hop1.ins)
                for el in range(4):
                    hop2 = nc.sync.dma_start(
                        scr2_d.ap()[:, el:el + 1, :, :].rearrange(
                            "jl el c g -> jl (el c) g"),
                        scr1_d.ap()[:, el:el + 1, :, :].rearrange(
                            "c el jl g -> jl (el c) g"))
                    tc.chain_iter_dep(f"yb_el{el}", hop2.ins)
                hop3 = nc.sync.dma_start(
                    ybias[:],
                    scr2_d.ap().rearrange("jl el c g -> (jl el) (c g)"))
                for el in range(4):
                    tc.chain_iter_dep(f"yb_el{el}", hop3.ins)
                # interleaved-j slices (matmul weights need plain 2-D APs)
                for t_ in range(NB):
                    nc.scalar.copy(
                        yTsel[:, t_ * 128:(t_ + 1) * 128].rearrange(
                            "p (q jl) -> p q jl", q=4), jsel(yT, t_))
                    nc.vector.tensor_copy(
                        xTsel[:, t_ * 128:(t_ + 1) * 128].rearrange(
                            "p (q jl) -> p q jl", q=4), jsel(xT, t_))
                # B_j for tile t rows (amask holds the linear coef per e)
                tb = pp.tile([128, 512], F32, tag="t")
                for t_ in range(NB):
                    nc.tensor.matmul(tb[:, t_:t_ + 1],
                                     yTsel[:, t_ * 128:(t_ + 1) * 128],
                                     amask[:], start=True, stop=True)
                nc.scalar.copy(Bsb[:], tb[:, 0:NB])
                # h (fp16), row-interleaved to match j(t, p)
                th = pp.tile([128, 512], F32, tag="t")
                for t_ in range(NB):
                    nc.tensor.matmul(th[:, t_ * D:(t_ + 1) * D],
                                     xTsel[:, t_ * 128:(t_ + 1) * 128], wp[:],
                                     start=True, stop=True)
                    if use_bproj:
                        nc.vector.tensor_tensor(
                            th[:, t_ * D:(t_ + 1) * D],
                            th[:, t_ * D:(t_ + 1) * D], bpr[:],
                            op=ALU.add)
                    nc.scalar.copy(h16[t_][:], th[:, t_ * D:(t_ + 1) * D])

            # ---------- main loop ----------
            with tc.tile_pool(name="zp", bufs=3, space="PSUM") as zpp, \
                 tc.tile_pool(name="sc", bufs=1, space="PSUM") as scp, \
                 tc.tile_pool(name="lp", bufs=16) as lp, \
                 tc.tile_pool(name="z4", bufs=4) as z4p:
                scores = [scp.tile([128, N], F32, tag=f"s{t}", name=f"s{t}")
                          for t in range(NB)]
                zp_tiles = {}

                def emit_repl(c):
                    zp = zpp.tile([128, 512], F32, tag="zp")
                    nc.tensor.matmul(zp[:], rep[:, c * 128:(c + 1) * 128],
                                     ziT[:], start=True, stop=True)
                    zp_tiles[c] = zp

                emit_repl(0)
                emit_repl(1)
                for c in range(NC_CHUNK):
                    if c + 2 < NC_CHUNK:
                        emit_repl(c + 2)
                    zp = zp_tiles.pop(c)
                    if MODE[c] == 'D':
                        z4 = z4p.tile([128, 512], F16, tag="z4")
                        nc.scalar.copy(z4[:], zp[:])
                    for g in range(NG):
                        col = c * NG + g
                        L = lp.tile([128, 512], F16, tag="L")
                        if MODE[c] == 'S':
                            nc.scalar.activation(L[:], zp[:], AF.Abs,
                                                 bias=ybias[:, col:col + 1],
                                                 scale=1.0)
                        else:
                            nc.vector.tensor_scalar(
                                L[:], z4[:], ybias[:, col:col + 1], 0.0,
                                op0=ALU.add, op1=ALU.max)
                        t_, q = g // 4, g % 4
                        nc.tensor.matmul(
                            scores[t_][32 * q:32 * (q + 1), :],
                            stat[:, c * 32:(c + 1) * 32], L[:],
                            start=(c == 0), stop=(c == NC_CHUNK - 1),
                            tile_position=(0, 32 * q))

                # ---------- softmax (transposed) ----------
                for t_ in range(NB):
                    nc.scalar.activation(expT[t_][:], scores[t_][:], AF.Exp,
                                         bias=Bsb[:, t_:t_ + 1], scale=1.0)

            # ---------- epilogue ----------
            with tc.tile_pool(name="ep", bufs=2, space="PSUM") as ep:
                zsp = ep.tile([128, 512], F32, tag="zs")
                for t_ in range(NB):
                    nc.tensor.matmul(zsp[0:1, :], ones[:], expT[t_][:],
                                     start=(t_ == 0), stop=(t_ == NB - 1))
                nc.scalar.copy(zsum_sb[:], zsp[0:1, :])
                rzp = ep.tile([128, 512], F32, tag="zs")
                for ib in range(NB):
                    nc.tensor.matmul(rzp[:, ib:ib + 1],
                                     zsum_sb[0:1, ib * 128:(ib + 1) * 128],
                                     one11[:], start=True, stop=True)
                nc.vector.reciprocal(rz[:], rzp[:, 0:NB])
                oall = sb.tile([128, NB * D], F32)
                for ib in range(NB):
                    acc = ep.tile([128, D], F32, tag="acc")
                    for t_ in range(NB):
                        nc.tensor.matmul(acc[:],
                                         expT[t_][:, ib * 128:(ib + 1) * 128],
                                         h16[t_][:],
                                         start=(t_ == 0), stop=(t_ == NB - 1))
                    nc.scalar.activation(oall[:, ib * D:(ib + 1) * D], acc[:],
                                         AF.Copy, bias=0.0,
                                         scale=rz[:, ib:ib + 1])
                    if use_bias_param:
                        nc.gpsimd.tensor_tensor(oall[:, ib * D:(ib + 1) * D],
                                                oall[:, ib * D:(ib + 1) * D],
                                                bprm[:], op=ALU.add)
                nc.sync.dma_start(
                    out_d.ap().rearrange("(ib n) d -> n ib d", ib=NB),
                    oall[:].rearrange("n (ib d) -> n ib d", ib=NB))

    nc.compile()
    return nc


def kernel(x, W_proj, b_proj, W_cat_weight, W_cat_bias, a, bias_param):
    global last_results
    x = np.asarray(x, dtype=np.float32)
    W_proj = np.asarray(W_proj, dtype=np.float32)
    b_proj = np.asarray(b_proj, dtype=np.float32)
    W_cat_weight = np.asarray(W_cat_weight, dtype=np.float32)
    W_cat_bias = np.asarray(W_cat_bias, dtype=np.float32)
    a = np.asarray(a, dtype=np.float32)
    bias_param = np.asarray(bias_param, dtype=np.float32)

    W1 = W_cat_weight[:, :, :D]
    W2 = W_cat_weight[:, :, D:]

    use_bproj = bool(np.any(b_proj))
    use_bias_param = bool(np.any(bias_param))
    key = (use_bproj, use_bias_param)
    if key not in _cache:
        _cache[key] = _build(*key)
    nc = _cache[key]

    # constant patterns (shared across cores)
    rep = np.zeros((E, NC_CHUNK * 128), dtype=np.float32)
    for c in range(NC_CHUNK):
        for p in range(128):
            rep[4 * c + p % 4, c * 128 + p] = 1.0
    rep = rep.astype(F16NP)
    id128 = np.eye(128, dtype=np.float32)
    ones = np.ones((128, 1), dtype=F16NP)
    one11 = np.ones((1, 1), dtype=np.float32)

    in_maps = []
    for core in range(8):
        b, hh = divmod(core, H)
        M1 = W1[hh] @ W_proj[hh].T          # (E, FIN)
        M2 = W2[hh] @ W_proj[hh].T          # (E, FIN)
        zibv = (W1[hh] @ b_proj[hh])[:, None].astype(np.float32)
        ybbv = (W_cat_bias[hh] + W2[hh] @ b_proj[hh])[:, None].astype(np.float32)
        ah = a[hh]
        # stat_c[jl*4+el, jl] = stationary coef for e = 4c+el
        statv = np.zeros((128, NC_CHUNK * 32), dtype=np.float32)
        amaskv = np.zeros((E, 1), dtype=np.float32)
        for c in range(NC_CHUNK):
            for el in range(4):
                e = 4 * c + el
                if MODE[c] == 'S':   # abs path
                    coef = (1.0 - NEG_SLOPE) / 2.0 * ah[e]      # 0.4 a
                    amaskv[e, 0] = (1.0 + NEG_SLOPE) / 2.0 * ah[e]  # 0.6 a
                else:                # relu path
                    coef = (1.0 - NEG_SLOPE) * ah[e]            # 0.8 a
                    amaskv[e, 0] = NEG_SLOPE * ah[e]            # 0.2 a
                for jl in range(32):
                    statv[jl * 4 + el, c * 32 + jl] = coef
        m = {
            "x": np.ascontiguousarray(x[b]),
            "m1t": np.ascontiguousarray(M1.T),
            "m2t": np.ascontiguousarray(M2.T),
            "zib": zibv,
            "ybb": ybbv,
            "amask": amaskv,
            "wp": np.ascontiguousarray(W_proj[hh]),
            "rep": rep,
            "stat": statv.astype(F16NP),
            "id128": id128,
            "ones": ones,
            "one11": one11,
        }
        if use_bproj:
            m["bprojrep"] = np.tile(b_proj[hh][None, :], (128, 1)).astype(np.float32)
        if use_bias_param:
            m["biasprm"] = np.tile(bias_param[None, hh * D:(hh + 1) * D],
                                   (128, 1)).astype(np.float32)
        in_maps.append(m)

    res = run_bass_kernel_spmd(nc, in_maps, core_ids=list(range(8)))
    last_results = res

    out = np.empty((B, N, H * D), dtype=np.float32)
    for core in range(8):
        b, hh = divmod(core, H)
        out[b, :, hh * D:(hh + 1) * D] = res.results[core]["out"]
    return out
